# revision 9
# baseline (speedup 1.0000x reference)
"""Trainium2 Bass kernel for nn_BoardEncoder (HexConv board encoder).

Math:
  h[b,n,:] = relu(x[b,n] @ Wc.T + sum_k neighbors[b,n,k] @ Wd[k].T + bc + bd.sum(0))
  out[b]   = h[b].reshape(216) @ Wf.T + bf

Strategy (pure data-parallel over batch, 8 cores x 2048 rows):
  - All stage-1 data in bf16: halves HBM traffic (the roofline) and runs
    the PE at 1 cycle/row instead of fp32's 4.
  - Host packs per-(b,n) token features [x | neighbors] (448 feats, the
    +bias moves into the relu) feature-major so the contraction dim sits
    on 112 SBUF partitions, K split into 4 chunks of 112.
  - 112 = 16*7 matters: SWDGE spreads a dma_start's per-partition
    descriptors evenly over the 16 SDMA engines only when the count
    divides; 113 (prime) serialized the whole load on one engine at
    ~23 GB/s.  One 5.25 MB dma_start per 3-cell group -> 16 engines x 7
    descriptors of 48 KB.
  - Stage 1 (per board cell n): psum[4, 512b] += Wchunk.T @ xtchunk
    (weights stationary across the 4 bt matmuls of a chunk), relu+bias
    on alternating vector/scalar engines -> bf16 strip [4, 2048],
    SBUF->SBUF DMA scatters to partition 4n of the h^T [(n,h), b]
    accumulator.
  - Stage 2: out[128b, 256] = hA.T @ WfT[:128] + hB.T @ WfT[128:] with a
    constant ones-row in hB providing the bf bias.
"""

import sys

sys.path.insert(0, "/opt/trn_rl_repo")

import numpy as np

B = 16384
N = 54
D_IN = 64
KN = 6
D_HID = 4
D_OUT = 256
NCORES = 8
BS = B // NCORES          # 2048 batch rows per core
F = D_IN + KN * D_IN      # 448 features (bias handled in the relu)
CH = 112                  # K-chunk partition size (4 * 112 = 448)
NCH = 4
GROUP = 3                 # board cells per DMA load
NG = N // GROUP           # 18 groups
BT = 512                  # stage-1 moving free dim (PSUM bank limit, fp32)
NBT = BS // BT            # 4

LAST_EXEC_NS = None

_PROGRAM = None


def _build_program(reps=1):
    import concourse.bacc as bacc
    import concourse.tile as tile
    from concourse import mybir

    f32 = mybir.dt.float32
    bf16 = mybir.dt.bfloat16
    ADD = mybir.AluOpType.add
    MAX = mybir.AluOpType.max

    nc = bacc.Bacc("TRN2", target_bir_lowering=False, debug=False,
                   num_devices=NCORES)
    xt_d = nc.declare_dram_parameter("xt", [NG, CH, GROUP * NCH * BS], bf16,
                                     isOutput=False)
    w_d = nc.declare_dram_parameter("w", [CH, NCH * D_HID], bf16,
                                    isOutput=False)
    bias_d = nc.declare_dram_parameter("biash", [D_HID, 1], f32,
                                       isOutput=False)
    wfta_d = nc.declare_dram_parameter("wfta", [128, D_OUT], bf16,
                                       isOutput=False)
    wftb_d = nc.declare_dram_parameter("wftb", [89, D_OUT], bf16,
                                       isOutput=False)
    out_d = nc.declare_dram_parameter("out", [BS, D_OUT], f32, isOutput=True)

    with tile.TileContext(nc) as tc:
        with (
            tc.tile_pool(name="consts", bufs=1) as consts,
            tc.tile_pool(name="hacc", bufs=1) as hacc,
            tc.tile_pool(name="xt", bufs=3) as xtp,
            tc.tile_pool(name="hn", bufs=6) as hnp,
            tc.tile_pool(name="ps1", bufs=6, space="PSUM") as ps1,
            tc.tile_pool(name="ps2", bufs=2, space="PSUM") as ps2,
            tc.tile_pool(name="outp", bufs=3) as outp,
        ):
            w_sb = consts.tile([CH, NCH * D_HID], bf16, tag="w")
            nc.sync.dma_start(w_sb[:], w_d[:])
            bias_sb = consts.tile([D_HID, 1], f32, tag="biash")
            nc.sync.dma_start(bias_sb[:], bias_d[:])
            wfta_sb = consts.tile([128, D_OUT], bf16, tag="wfta")
            nc.sync.dma_start(wfta_sb[:], wfta_d[:])
            wftb_sb = consts.tile([89, D_OUT], bf16, tag="wftb")
            nc.sync.dma_start(wftb_sb[:], wftb_d[:])

            for rep in range(reps):
                hA = hacc.tile([128, BS], bf16, tag="hA")  # (n,h) rows 0..127
                hB = hacc.tile([89, BS], bf16, tag="hB")   # rows 128..215+ones
                # rows 0..87 are overwritten by the per-cell scatter DMAs
                # below; row 88 keeps the 1.0 fill and provides the bf bias
                # in stage 2. (a [88:89] memset is rejected: compute-engine
                # partition bases must be 32-aligned)
                nc.vector.memset(hB[:, :], 1.0)

                def scatter(n, hn):
                    # scatter on sync/HWDGE so its sem-wait on the relu does
                    # not block the big-input-load FIFO (loads live on the
                    # gpsimd/SWDGE path)
                    if n < 32:
                        nc.sync.dma_start(hA[n * 4:(n + 1) * 4, :], hn[:])
                    else:
                        m = n - 32
                        nc.sync.dma_start(hB[m * 4:(m + 1) * 4, :], hn[:])

                for g in range(NG):
                    xt = xtp.tile([CH, GROUP * NCH * BS], bf16)
                    # Partition-slice each load into prime-sized (>16 would
                    # not divide) chunks: SWDGE assigns each dma_start's
                    # descriptors to ONE engine (rotating per dma_start), so
                    # every engine reads a ~912KB CONTIGUOUS DRAM stream at
                    # ~23GB/s instead of 768KB-strided descriptors at
                    # ~14GB/s (HBM row locality).
                    for p0, p1 in ((0, 19), (19, 38), (38, 57), (57, 76),
                                   (76, 95), (95, CH)):
                        nc.gpsimd.dma_start(xt[p0:p1, :], xt_d[g, p0:p1, :])
                    for j in range(GROUP):
                        hn = hnp.tile([D_HID, BS], bf16, name="hn")
                        pss = [ps1.tile([D_HID, BT], f32, name="ps")
                               for _ in range(NBT)]
                        # weights stationary across the 4 bt matmuls of a
                        # chunk: 4 loads per cell instead of 16
                        for c in range(NCH):
                            for bt in range(NBT):
                                col = j * NCH * BS + c * BS + bt * BT
                                nc.tensor.matmul(
                                    pss[bt][:],
                                    w_sb[:, c * D_HID:(c + 1) * D_HID],
                                    xt[:, col:col + BT],
                                    start=(c == 0),
                                    stop=(c == NCH - 1),
                                )
                        for bt in range(NBT):
                            dst = hn[:, bt * BT:(bt + 1) * BT]
                            src = pss[bt][:]
                            if bt % 2 == 0:
                                nc.vector.tensor_scalar(
                                    dst, src, bias_sb[:, 0:1], 0.0, ADD, MAX)
                            else:
                                nc.scalar.activation(
                                    dst, src,
                                    mybir.ActivationFunctionType.Relu,
                                    bias=bias_sb[:, 0:1])
                        scatter(GROUP * g + j, hn)

                for t in range(BS // 128):
                    po = ps2.tile([128, D_OUT], f32, name="po",
                                  padded_shape=[128, 512])
                    nc.tensor.matmul(po[:], hA[:, t * 128:(t + 1) * 128],
                                     wfta_sb[:], start=True, stop=False)
                    nc.tensor.matmul(po[:], hB[:, t * 128:(t + 1) * 128],
                                     wftb_sb[:], start=False, stop=True)
                    ot = outp.tile([128, D_OUT], f32)
                    if t % 2 == 0:
                        nc.vector.tensor_copy(ot[:], po[:])
                    else:
                        nc.scalar.copy(ot[:], po[:])
                    nc.sync.dma_start(out_d[t * 128:(t + 1) * 128, :], ot[:])

    nc.compile()
    return nc


def _get_program():
    global _PROGRAM
    if _PROGRAM is None:
        _PROGRAM = _build_program()
    return _PROGRAM


def _to_bf16(a):
    """fp32 -> bf16 with round-to-nearest-even, via integer ops (fast)."""
    import ml_dtypes

    u = np.ascontiguousarray(a).view(np.uint32)
    u16 = ((u + np.uint32(0x7FFF) + ((u >> np.uint32(16)) & np.uint32(1)))
           >> np.uint32(16)).astype(np.uint16)
    return u16.view(ml_dtypes.bfloat16)


def _pack_inputs(x, neighbors):
    """xt[s, g, p, j*8192 + c*2048 + b] = feat[c*112 + p] of batch row
    (s*BS + b), cell GROUP*g+j. feat = [x | neighbors], bf16."""
    feats = np.empty((B, N, F), np.float32)
    feats[:, :, :D_IN] = x
    feats[:, :, D_IN:] = neighbors.reshape(B, N, KN * D_IN)
    bf = _to_bf16(feats)
    del feats
    t = bf.reshape(NCORES, BS, NG, GROUP, NCH, CH).transpose(0, 2, 5, 3, 4, 1)
    return np.ascontiguousarray(t).reshape(NCORES, NG, CH, GROUP * NCH * BS)


def _pack_weights(Wc, bc, Wd, bd, Wf, bf):
    W_all = np.empty((F, D_HID), np.float32)
    W_all[:D_IN] = Wc.T
    W_all[D_IN:] = Wd.transpose(0, 2, 1).reshape(KN * D_IN, D_HID)
    # w[p, c*4+h] = W_all[c*112+p, h]
    w = _to_bf16(np.ascontiguousarray(
        W_all.reshape(NCH, CH, D_HID).transpose(1, 0, 2)).reshape(
            CH, NCH * D_HID))
    bias_h = np.ascontiguousarray(
        (bc + bd.sum(0)).reshape(D_HID, 1).astype(np.float32))
    WfT = np.ascontiguousarray(Wf.T)            # [216, 256]
    wfta = _to_bf16(np.ascontiguousarray(WfT[:128]))
    wftb = _to_bf16(np.ascontiguousarray(
        np.concatenate([WfT[128:], bf[None, :]], axis=0)))  # [89, 256]
    return w, bias_h, wfta, wftb


def _spot_check(out, xts, w, bias_h, wfta, wftb):
    """Cheap numpy recompute of sampled batch rows from the same packed
    bf16 data the device consumed. Catches transient device corruption
    (seen once on a cold first-execution); expected rel err ~0.5%/row."""
    bsel = np.arange(0, BS, 32)                        # 64 rows per shard
    wp = np.asarray(w, np.float32).reshape(CH, NCH, D_HID)
    wfull = np.concatenate([np.asarray(wfta, np.float32),
                            np.asarray(wftb, np.float32)], axis=0)  # [217,256]
    cols = (np.arange(GROUP)[:, None, None] * NCH * BS
            + np.arange(NCH)[None, :, None] * BS
            + bsel[None, None, :])                     # [GROUP, NCH, nb]
    worst = 0.0
    for s in range(NCORES):
        g = np.asarray(xts[s][:, :, cols.reshape(-1)], np.float32)
        g = g.reshape(NG, CH, GROUP, NCH, len(bsel))
        pre = np.einsum('npjcb,pch->njhb', g, wp, optimize=True)
        h = np.maximum(pre + bias_h.reshape(1, 1, D_HID, 1), 0.0)
        h = np.asarray(_to_bf16(np.ascontiguousarray(h, np.float32)),
                       np.float32)
        flat = h.reshape(N * D_HID, len(bsel))         # rows = (n,h)
        ref = flat.T @ wfull[:216] + wfull[216]
        got = out[s * BS + bsel]
        num = np.linalg.norm(got - ref, axis=1)
        den = np.linalg.norm(ref, axis=1) + 1e-6
        worst = max(worst, float(np.max(num / den)))
    return worst


def kernel(x, neighbors, Wc, bc, Wd, bd, Wf, bf):
    global LAST_EXEC_NS
    from concourse.bass_utils import run_bass_kernel_spmd

    x = np.asarray(x, np.float32)
    neighbors = np.asarray(neighbors, np.float32)
    w, bias_h, wfta, wftb = _pack_weights(
        np.asarray(Wc, np.float32), np.asarray(bc, np.float32),
        np.asarray(Wd, np.float32), np.asarray(bd, np.float32),
        np.asarray(Wf, np.float32), np.asarray(bf, np.float32))
    xts = _pack_inputs(x, neighbors)

    nc = _get_program()
    in_maps = [
        {"xt": xts[s], "w": w, "biash": bias_h, "wfta": wfta, "wftb": wftb}
        for s in range(NCORES)
    ]
    for attempt in range(3):
        res = run_bass_kernel_spmd(nc, in_maps, list(range(NCORES)))
        LAST_EXEC_NS = res.exec_time_ns
        out = np.concatenate([res.results[s]["out"] for s in range(NCORES)],
                             axis=0)
        if _spot_check(out, xts, w, bias_h, wfta, wftb) < 0.05:
            break
    return out


# revision 11
# speedup vs baseline: 1.5245x; 1.5245x over previous
"""Trainium2 Bass kernel for nn_BoardEncoder (HexConv board encoder).

Math:
  h[b,n,:] = relu(x[b,n] @ Wc.T + sum_k neighbors[b,n,k] @ Wd[k].T + bc + bd.sum(0))
  out[b]   = h[b].reshape(216) @ Wf.T + bf

Strategy (pure data-parallel over batch, 8 cores x 2048 rows):
  - All stage-1 data in bf16: halves HBM traffic (the roofline) and runs
    the PE at 1 cycle/row instead of fp32's 4.
  - Host packs per-(b,n) token features [x | neighbors] (448 feats, the
    +bias moves into the relu) feature-major so the contraction dim sits
    on 112 SBUF partitions, K split into 4 chunks of 112.
  - 112 = 16*7 matters: SWDGE spreads a dma_start's per-partition
    descriptors evenly over the 16 SDMA engines only when the count
    divides; 113 (prime) serialized the whole load on one engine at
    ~23 GB/s.  One 5.25 MB dma_start per 3-cell group -> 16 engines x 7
    descriptors of 48 KB.
  - Stage 1 (per board cell n): psum[4, 512b] += Wchunk.T @ xtchunk
    (weights stationary across the 4 bt matmuls of a chunk), relu+bias
    on alternating vector/scalar engines -> bf16 strip [4, 2048],
    SBUF->SBUF DMA scatters to partition 4n of the h^T [(n,h), b]
    accumulator.
  - Stage 2: out[128b, 256] = hA.T @ WfT[:128] + hB.T @ WfT[128:] with a
    constant ones-row in hB providing the bf bias.
"""

import sys

sys.path.insert(0, "/opt/trn_rl_repo")

import numpy as np

B = 16384
N = 54
D_IN = 64
KN = 6
D_HID = 4
D_OUT = 256
NCORES = 8
BS = B // NCORES          # 2048 batch rows per core
F = D_IN + KN * D_IN      # 448 features (bias handled in the relu)
CH = 112                  # K-chunk partition size (4 * 112 = 448)
NCH = 4
GROUP = 2                 # board cells per DMA load
NG = N // GROUP           # 27 groups
BT = 512                  # stage-1 moving free dim (PSUM bank limit, fp32)
NBT = BS // BT            # 4

LAST_EXEC_NS = None

_PROGRAM = None


def _build_program(reps=1):
    import concourse.bacc as bacc
    import concourse.tile as tile
    from concourse import mybir

    f32 = mybir.dt.float32
    bf16 = mybir.dt.bfloat16
    ADD = mybir.AluOpType.add
    MAX = mybir.AluOpType.max

    nc = bacc.Bacc("TRN2", target_bir_lowering=False, debug=False,
                   num_devices=NCORES)
    xt_d = nc.declare_dram_parameter("xt", [NG, CH, GROUP * NCH * BS], bf16,
                                     isOutput=False)
    w_d = nc.declare_dram_parameter("w", [CH, NCH * D_HID], bf16,
                                    isOutput=False)
    bias_d = nc.declare_dram_parameter("biash", [D_HID, 1], f32,
                                       isOutput=False)
    wfta_d = nc.declare_dram_parameter("wfta", [128, D_OUT], bf16,
                                       isOutput=False)
    wftb_d = nc.declare_dram_parameter("wftb", [89, D_OUT], bf16,
                                       isOutput=False)
    out_d = nc.declare_dram_parameter("out", [BS, D_OUT], f32, isOutput=True)

    with tile.TileContext(nc) as tc:
        with (
            tc.tile_pool(name="consts", bufs=1) as consts,
            tc.tile_pool(name="hacc", bufs=1) as hacc,
            tc.tile_pool(name="xt", bufs=3) as xtp,
            tc.tile_pool(name="hn", bufs=6) as hnp,
            tc.tile_pool(name="ps1", bufs=6, space="PSUM") as ps1,
            tc.tile_pool(name="ps2", bufs=2, space="PSUM") as ps2,
            tc.tile_pool(name="outp", bufs=3) as outp,
        ):
            w_sb = consts.tile([CH, NCH * D_HID], bf16, tag="w")
            nc.sync.dma_start(w_sb[:], w_d[:])
            bias_sb = consts.tile([D_HID, 1], f32, tag="biash")
            nc.sync.dma_start(bias_sb[:], bias_d[:])
            wfta_sb = consts.tile([128, D_OUT], bf16, tag="wfta")
            nc.sync.dma_start(wfta_sb[:], wfta_d[:])
            wftb_sb = consts.tile([89, D_OUT], bf16, tag="wftb")
            nc.sync.dma_start(wftb_sb[:], wftb_d[:])

            for rep in range(reps):
                hA = hacc.tile([128, BS], bf16, tag="hA")  # (n,h) rows 0..127
                hB = hacc.tile([89, BS], bf16, tag="hB")   # rows 128..215+ones
                # rows 0..87 are overwritten by the per-cell scatter DMAs
                # below; row 88 keeps the 1.0 fill and provides the bf bias
                # in stage 2. (a [88:89] memset is rejected: compute-engine
                # partition bases must be 32-aligned)
                nc.vector.memset(hB[:, :], 1.0)

                def scatter(n, hn):
                    # scatter on sync/HWDGE so its sem-wait on the relu does
                    # not block the big-input-load FIFO (loads live on the
                    # gpsimd/SWDGE path)
                    if n < 32:
                        nc.sync.dma_start(hA[n * 4:(n + 1) * 4, :], hn[:])
                    else:
                        m = n - 32
                        nc.sync.dma_start(hB[m * 4:(m + 1) * 4, :], hn[:])

                for g in range(NG):
                    xt = xtp.tile([CH, GROUP * NCH * BS], bf16)
                    nc.gpsimd.dma_start(xt[:], xt_d[g])
                    for j in range(GROUP):
                        hn = hnp.tile([D_HID, BS], bf16, name="hn")
                        pss = [ps1.tile([D_HID, BT], f32, name="ps")
                               for _ in range(NBT)]
                        # weights stationary across the 4 bt matmuls of a
                        # chunk: 4 loads per cell instead of 16
                        for c in range(NCH):
                            for bt in range(NBT):
                                col = j * NCH * BS + c * BS + bt * BT
                                nc.tensor.matmul(
                                    pss[bt][:],
                                    w_sb[:, c * D_HID:(c + 1) * D_HID],
                                    xt[:, col:col + BT],
                                    start=(c == 0),
                                    stop=(c == NCH - 1),
                                )
                        for bt in range(NBT):
                            dst = hn[:, bt * BT:(bt + 1) * BT]
                            src = pss[bt][:]
                            if bt % 2 == 0:
                                nc.vector.tensor_scalar(
                                    dst, src, bias_sb[:, 0:1], 0.0, ADD, MAX)
                            else:
                                nc.scalar.activation(
                                    dst, src,
                                    mybir.ActivationFunctionType.Relu,
                                    bias=bias_sb[:, 0:1])
                        scatter(GROUP * g + j, hn)

                for t in range(BS // 128):
                    po = ps2.tile([128, D_OUT], f32, name="po",
                                  padded_shape=[128, 512])
                    nc.tensor.matmul(po[:], hA[:, t * 128:(t + 1) * 128],
                                     wfta_sb[:], start=True, stop=False)
                    nc.tensor.matmul(po[:], hB[:, t * 128:(t + 1) * 128],
                                     wftb_sb[:], start=False, stop=True)
                    ot = outp.tile([128, D_OUT], f32)
                    if t % 2 == 0:
                        nc.vector.tensor_copy(ot[:], po[:])
                    else:
                        nc.scalar.copy(ot[:], po[:])
                    nc.sync.dma_start(out_d[t * 128:(t + 1) * 128, :], ot[:])

    nc.compile()
    return nc


def _get_program():
    global _PROGRAM
    if _PROGRAM is None:
        _PROGRAM = _build_program()
    return _PROGRAM


def _to_bf16(a):
    """fp32 -> bf16 with round-to-nearest-even, via integer ops (fast)."""
    import ml_dtypes

    u = np.ascontiguousarray(a).view(np.uint32)
    u16 = ((u + np.uint32(0x7FFF) + ((u >> np.uint32(16)) & np.uint32(1)))
           >> np.uint32(16)).astype(np.uint16)
    return u16.view(ml_dtypes.bfloat16)


def _pack_inputs(x, neighbors):
    """xt[s, g, p, j*8192 + c*2048 + b] = feat[c*112 + p] of batch row
    (s*BS + b), cell GROUP*g+j. feat = [x | neighbors], bf16."""
    feats = np.empty((B, N, F), np.float32)
    feats[:, :, :D_IN] = x
    feats[:, :, D_IN:] = neighbors.reshape(B, N, KN * D_IN)
    bf = _to_bf16(feats)
    del feats
    t = bf.reshape(NCORES, BS, NG, GROUP, NCH, CH).transpose(0, 2, 5, 3, 4, 1)
    return np.ascontiguousarray(t).reshape(NCORES, NG, CH, GROUP * NCH * BS)


def _pack_weights(Wc, bc, Wd, bd, Wf, bf):
    W_all = np.empty((F, D_HID), np.float32)
    W_all[:D_IN] = Wc.T
    W_all[D_IN:] = Wd.transpose(0, 2, 1).reshape(KN * D_IN, D_HID)
    # w[p, c*4+h] = W_all[c*112+p, h]
    w = _to_bf16(np.ascontiguousarray(
        W_all.reshape(NCH, CH, D_HID).transpose(1, 0, 2)).reshape(
            CH, NCH * D_HID))
    bias_h = np.ascontiguousarray(
        (bc + bd.sum(0)).reshape(D_HID, 1).astype(np.float32))
    WfT = np.ascontiguousarray(Wf.T)            # [216, 256]
    wfta = _to_bf16(np.ascontiguousarray(WfT[:128]))
    wftb = _to_bf16(np.ascontiguousarray(
        np.concatenate([WfT[128:], bf[None, :]], axis=0)))  # [89, 256]
    return w, bias_h, wfta, wftb


def _spot_check(out, xts, w, bias_h, wfta, wftb):
    """Cheap numpy recompute of sampled batch rows from the same packed
    bf16 data the device consumed. Catches transient device corruption
    (seen once on a cold first-execution); expected rel err ~0.5%/row."""
    bsel = np.arange(0, BS, 32)                        # 64 rows per shard
    wp = np.asarray(w, np.float32).reshape(CH, NCH, D_HID)
    wfull = np.concatenate([np.asarray(wfta, np.float32),
                            np.asarray(wftb, np.float32)], axis=0)  # [217,256]
    cols = (np.arange(GROUP)[:, None, None] * NCH * BS
            + np.arange(NCH)[None, :, None] * BS
            + bsel[None, None, :])                     # [GROUP, NCH, nb]
    worst = 0.0
    for s in range(NCORES):
        g = np.asarray(xts[s][:, :, cols.reshape(-1)], np.float32)
        g = g.reshape(NG, CH, GROUP, NCH, len(bsel))
        pre = np.einsum('npjcb,pch->njhb', g, wp, optimize=True)
        h = np.maximum(pre + bias_h.reshape(1, 1, D_HID, 1), 0.0)
        h = np.asarray(_to_bf16(np.ascontiguousarray(h, np.float32)),
                       np.float32)
        flat = h.reshape(N * D_HID, len(bsel))         # rows = (n,h)
        ref = flat.T @ wfull[:216] + wfull[216]
        got = out[s * BS + bsel]
        num = np.linalg.norm(got - ref, axis=1)
        den = np.linalg.norm(ref, axis=1) + 1e-6
        worst = max(worst, float(np.max(num / den)))
    return worst


def kernel(x, neighbors, Wc, bc, Wd, bd, Wf, bf):
    global LAST_EXEC_NS
    from concourse.bass_utils import run_bass_kernel_spmd

    x = np.asarray(x, np.float32)
    neighbors = np.asarray(neighbors, np.float32)
    w, bias_h, wfta, wftb = _pack_weights(
        np.asarray(Wc, np.float32), np.asarray(bc, np.float32),
        np.asarray(Wd, np.float32), np.asarray(bd, np.float32),
        np.asarray(Wf, np.float32), np.asarray(bf, np.float32))
    xts = _pack_inputs(x, neighbors)

    nc = _get_program()
    in_maps = [
        {"xt": xts[s], "w": w, "biash": bias_h, "wfta": wfta, "wftb": wftb}
        for s in range(NCORES)
    ]
    for attempt in range(3):
        res = run_bass_kernel_spmd(nc, in_maps, list(range(NCORES)))
        LAST_EXEC_NS = res.exec_time_ns
        out = np.concatenate([res.results[s]["out"] for s in range(NCORES)],
                             axis=0)
        if _spot_check(out, xts, w, bias_h, wfta, wftb) < 0.05:
            break
    return out


# revision 12
# speedup vs baseline: 1.6939x; 1.1111x over previous
"""Trainium2 Bass kernel for nn_BoardEncoder (HexConv board encoder).

Math:
  h[b,n,:] = relu(x[b,n] @ Wc.T + sum_k neighbors[b,n,k] @ Wd[k].T + bc + bd.sum(0))
  out[b]   = h[b].reshape(216) @ Wf.T + bf

Strategy (pure data-parallel over batch, 8 cores x 2048 rows):
  - All stage-1 data in bf16: halves HBM traffic (the roofline) and runs
    the PE at 1 cycle/row instead of fp32's 4.
  - Host packs per-(b,n) token features [x | neighbors] (448 feats, the
    +bias moves into the relu) feature-major so the contraction dim sits
    on 112 SBUF partitions, K split into 4 chunks of 112.
  - 112 = 16*7 matters: SWDGE spreads a dma_start's per-partition
    descriptors evenly over the 16 SDMA engines only when the count
    divides; 113 (prime) serialized the whole load on one engine at
    ~23 GB/s.  One 5.25 MB dma_start per 3-cell group -> 16 engines x 7
    descriptors of 48 KB.
  - Stage 1 (per board cell n): psum[4, 512b] += Wchunk.T @ xtchunk
    (weights stationary across the 4 bt matmuls of a chunk), relu+bias
    on alternating vector/scalar engines -> bf16 strip [4, 2048],
    SBUF->SBUF DMA scatters to partition 4n of the h^T [(n,h), b]
    accumulator.
  - Stage 2: out[128b, 256] = hA.T @ WfT[:128] + hB.T @ WfT[128:] with a
    constant ones-row in hB providing the bf bias.
"""

import sys

sys.path.insert(0, "/opt/trn_rl_repo")

import numpy as np

B = 16384
N = 54
D_IN = 64
KN = 6
D_HID = 4
D_OUT = 256
NCORES = 8
BS = B // NCORES          # 2048 batch rows per core
F = D_IN + KN * D_IN      # 448 features (bias handled in the relu)
CH = 112                  # K-chunk partition size (4 * 112 = 448)
NCH = 4
GROUP = 2                 # board cells per DMA load
NG = N // GROUP           # 27 groups
BT = 512                  # stage-1 moving free dim (PSUM bank limit, fp32)
NBT = BS // BT            # 4

LAST_EXEC_NS = None

_PROGRAM = None


def _build_program(reps=1):
    import concourse.bacc as bacc
    import concourse.tile as tile
    from concourse import mybir

    f32 = mybir.dt.float32
    bf16 = mybir.dt.bfloat16
    ADD = mybir.AluOpType.add
    MAX = mybir.AluOpType.max

    nc = bacc.Bacc("TRN2", target_bir_lowering=False, debug=False,
                   num_devices=NCORES)
    xt_d = nc.declare_dram_parameter("xt", [NG, CH, GROUP * NCH * BS], bf16,
                                     isOutput=False)
    w_d = nc.declare_dram_parameter("w", [CH, NCH * D_HID], bf16,
                                    isOutput=False)
    bias_d = nc.declare_dram_parameter("biash", [D_HID, 1], f32,
                                       isOutput=False)
    wfta_d = nc.declare_dram_parameter("wfta", [128, D_OUT], bf16,
                                       isOutput=False)
    wftb_d = nc.declare_dram_parameter("wftb", [89, D_OUT], bf16,
                                       isOutput=False)
    out_d = nc.declare_dram_parameter("out", [BS, D_OUT], f32, isOutput=True)

    with tile.TileContext(nc) as tc:
        with (
            tc.tile_pool(name="consts", bufs=1) as consts,
            tc.tile_pool(name="hacc", bufs=1) as hacc,
            tc.tile_pool(name="xt", bufs=3) as xtp,
            tc.tile_pool(name="hn", bufs=6) as hnp,
            tc.tile_pool(name="ps1", bufs=6, space="PSUM") as ps1,
            tc.tile_pool(name="ps2", bufs=2, space="PSUM") as ps2,
            tc.tile_pool(name="outp", bufs=3) as outp,
        ):
            w_sb = consts.tile([CH, NCH * D_HID], bf16, tag="w")
            nc.sync.dma_start(w_sb[:], w_d[:])
            bias_sb = consts.tile([D_HID, 1], f32, tag="biash")
            nc.sync.dma_start(bias_sb[:], bias_d[:])
            wfta_sb = consts.tile([128, D_OUT], bf16, tag="wfta")
            nc.sync.dma_start(wfta_sb[:], wfta_d[:])
            wftb_sb = consts.tile([89, D_OUT], bf16, tag="wftb")
            nc.sync.dma_start(wftb_sb[:], wftb_d[:])

            for rep in range(reps):
                hA = hacc.tile([128, BS], bf16, tag="hA")  # (n,h) rows 0..127
                hB = hacc.tile([89, BS], bf16, tag="hB")   # rows 128..215+ones
                # rows 0..87 are overwritten by the per-cell scatter DMAs
                # below; row 88 keeps the 1.0 fill and provides the bf bias
                # in stage 2. (a [88:89] memset is rejected: compute-engine
                # partition bases must be 32-aligned)
                nc.vector.memset(hB[:, :], 1.0)

                def scatter(n, hn):
                    # scatter on scalar/HWDGE: its sem-wait on the relu
                    # resolves right after ACT's own relus, and it must not
                    # sit on the sync ring where it would block dispatch of
                    # the big input loads
                    if n < 32:
                        nc.scalar.dma_start(hA[n * 4:(n + 1) * 4, :], hn[:])
                    else:
                        m = n - 32
                        nc.scalar.dma_start(hB[m * 4:(m + 1) * 4, :], hn[:])

                half = GROUP * NCH * BS // 2
                for g in range(NG):
                    xt = xtp.tile([CH, GROUP * NCH * BS], bf16)
                    # split each load across the SWDGE (gpsimd) and HWDGE
                    # (sync) rings: each SDMA engine round-robins packets
                    # from both rings, doubling its outstanding work and
                    # masking HBM latency
                    nc.gpsimd.dma_start(xt[:, :half], xt_d[g, :, :half])
                    nc.sync.dma_start(xt[:, half:], xt_d[g, :, half:])
                    for j in range(GROUP):
                        hn = hnp.tile([D_HID, BS], bf16, name="hn")
                        pss = [ps1.tile([D_HID, BT], f32, name="ps")
                               for _ in range(NBT)]
                        # weights stationary across the 4 bt matmuls of a
                        # chunk: 4 loads per cell instead of 16
                        for c in range(NCH):
                            for bt in range(NBT):
                                col = j * NCH * BS + c * BS + bt * BT
                                nc.tensor.matmul(
                                    pss[bt][:],
                                    w_sb[:, c * D_HID:(c + 1) * D_HID],
                                    xt[:, col:col + BT],
                                    start=(c == 0),
                                    stop=(c == NCH - 1),
                                )
                        for bt in range(NBT):
                            dst = hn[:, bt * BT:(bt + 1) * BT]
                            src = pss[bt][:]
                            if bt % 2 == 0:
                                nc.vector.tensor_scalar(
                                    dst, src, bias_sb[:, 0:1], 0.0, ADD, MAX)
                            else:
                                nc.scalar.activation(
                                    dst, src,
                                    mybir.ActivationFunctionType.Relu,
                                    bias=bias_sb[:, 0:1])
                        scatter(GROUP * g + j, hn)

                for t in range(BS // 128):
                    po = ps2.tile([128, D_OUT], f32, name="po",
                                  padded_shape=[128, 512])
                    nc.tensor.matmul(po[:], hA[:, t * 128:(t + 1) * 128],
                                     wfta_sb[:], start=True, stop=False)
                    nc.tensor.matmul(po[:], hB[:, t * 128:(t + 1) * 128],
                                     wftb_sb[:], start=False, stop=True)
                    ot = outp.tile([128, D_OUT], f32)
                    if t % 2 == 0:
                        nc.vector.tensor_copy(ot[:], po[:])
                    else:
                        nc.scalar.copy(ot[:], po[:])
                    nc.sync.dma_start(out_d[t * 128:(t + 1) * 128, :], ot[:])

    nc.compile()
    return nc


def _get_program():
    global _PROGRAM
    if _PROGRAM is None:
        _PROGRAM = _build_program()
    return _PROGRAM


def _to_bf16(a):
    """fp32 -> bf16 with round-to-nearest-even, via integer ops (fast)."""
    import ml_dtypes

    u = np.ascontiguousarray(a).view(np.uint32)
    u16 = ((u + np.uint32(0x7FFF) + ((u >> np.uint32(16)) & np.uint32(1)))
           >> np.uint32(16)).astype(np.uint16)
    return u16.view(ml_dtypes.bfloat16)


def _pack_inputs(x, neighbors):
    """xt[s, g, p, j*8192 + c*2048 + b] = feat[c*112 + p] of batch row
    (s*BS + b), cell GROUP*g+j. feat = [x | neighbors], bf16."""
    feats = np.empty((B, N, F), np.float32)
    feats[:, :, :D_IN] = x
    feats[:, :, D_IN:] = neighbors.reshape(B, N, KN * D_IN)
    bf = _to_bf16(feats)
    del feats
    t = bf.reshape(NCORES, BS, NG, GROUP, NCH, CH).transpose(0, 2, 5, 3, 4, 1)
    return np.ascontiguousarray(t).reshape(NCORES, NG, CH, GROUP * NCH * BS)


def _pack_weights(Wc, bc, Wd, bd, Wf, bf):
    W_all = np.empty((F, D_HID), np.float32)
    W_all[:D_IN] = Wc.T
    W_all[D_IN:] = Wd.transpose(0, 2, 1).reshape(KN * D_IN, D_HID)
    # w[p, c*4+h] = W_all[c*112+p, h]
    w = _to_bf16(np.ascontiguousarray(
        W_all.reshape(NCH, CH, D_HID).transpose(1, 0, 2)).reshape(
            CH, NCH * D_HID))
    bias_h = np.ascontiguousarray(
        (bc + bd.sum(0)).reshape(D_HID, 1).astype(np.float32))
    WfT = np.ascontiguousarray(Wf.T)            # [216, 256]
    wfta = _to_bf16(np.ascontiguousarray(WfT[:128]))
    wftb = _to_bf16(np.ascontiguousarray(
        np.concatenate([WfT[128:], bf[None, :]], axis=0)))  # [89, 256]
    return w, bias_h, wfta, wftb


def _spot_check(out, xts, w, bias_h, wfta, wftb):
    """Cheap numpy recompute of sampled batch rows from the same packed
    bf16 data the device consumed. Catches transient device corruption
    (seen once on a cold first-execution); expected rel err ~0.5%/row."""
    bsel = np.arange(0, BS, 32)                        # 64 rows per shard
    wp = np.asarray(w, np.float32).reshape(CH, NCH, D_HID)
    wfull = np.concatenate([np.asarray(wfta, np.float32),
                            np.asarray(wftb, np.float32)], axis=0)  # [217,256]
    cols = (np.arange(GROUP)[:, None, None] * NCH * BS
            + np.arange(NCH)[None, :, None] * BS
            + bsel[None, None, :])                     # [GROUP, NCH, nb]
    worst = 0.0
    for s in range(NCORES):
        g = np.asarray(xts[s][:, :, cols.reshape(-1)], np.float32)
        g = g.reshape(NG, CH, GROUP, NCH, len(bsel))
        pre = np.einsum('npjcb,pch->njhb', g, wp, optimize=True)
        h = np.maximum(pre + bias_h.reshape(1, 1, D_HID, 1), 0.0)
        h = np.asarray(_to_bf16(np.ascontiguousarray(h, np.float32)),
                       np.float32)
        flat = h.reshape(N * D_HID, len(bsel))         # rows = (n,h)
        ref = flat.T @ wfull[:216] + wfull[216]
        got = out[s * BS + bsel]
        num = np.linalg.norm(got - ref, axis=1)
        den = np.linalg.norm(ref, axis=1) + 1e-6
        worst = max(worst, float(np.max(num / den)))
    return worst


def kernel(x, neighbors, Wc, bc, Wd, bd, Wf, bf):
    global LAST_EXEC_NS
    from concourse.bass_utils import run_bass_kernel_spmd

    x = np.asarray(x, np.float32)
    neighbors = np.asarray(neighbors, np.float32)
    w, bias_h, wfta, wftb = _pack_weights(
        np.asarray(Wc, np.float32), np.asarray(bc, np.float32),
        np.asarray(Wd, np.float32), np.asarray(bd, np.float32),
        np.asarray(Wf, np.float32), np.asarray(bf, np.float32))
    xts = _pack_inputs(x, neighbors)

    nc = _get_program()
    in_maps = [
        {"xt": xts[s], "w": w, "biash": bias_h, "wfta": wfta, "wftb": wftb}
        for s in range(NCORES)
    ]
    for attempt in range(3):
        res = run_bass_kernel_spmd(nc, in_maps, list(range(NCORES)))
        LAST_EXEC_NS = res.exec_time_ns
        out = np.concatenate([res.results[s]["out"] for s in range(NCORES)],
                             axis=0)
        if _spot_check(out, xts, w, bias_h, wfta, wftb) < 0.05:
            break
    return out
